# revision 39
# baseline (speedup 1.0000x reference)
"""Trainium2 Bass kernel: 16-head MHA (S=4096, D=1024) sharded 2 heads/core over 8 cores.

Per-core plan (heads h0=2c, h1=2c+1), v3 "round" architecture:
  - layouts: qT/xT [j, S] bf16 with j = h*64+dk on all 128 partitions.  kT
    stored per head as ZERO-PADDED K=128 tiles (kT0 rows 64:128 = 0, kT1 rows
    0:64 = 0): QK becomes a full-K matmul against the combined [h0|h1] qT (the
    zeros annihilate the other head) — HW-measured 163ns/MM vs 296ns at K=64.
    v in [t, (h, dk)] layout as vx [128, 32, 2, 65] fp16 (ones column 64 =
    softmax denominator).  v bias applied on host (exact: softmax rows sum to 1).
  - prologue: k and v projected for the full sequence (packed M=128 matmuls:
    both heads in one stationary tile), plus q for s-block 0.
  - steady state (per s-block, both heads together in 32 rounds of one t-block):
    round tb: QK h0 -> slot[:, 0, :], QK h1 -> slot[:, 1, :] (2-bank A/B
    ping-pong); ONE ScalarE exp covers both heads [128, 1024] psum -> fp16 pt
    tile (HW ~769ns); PV volleys (vx.T @ P per head, K=128) lag LAG=6 rounds.
    ScalarE is the bottleneck engine, so the remaining PE/DVE work rides in
    the slack as work items, one per round, emitted BEFORE the PV volley (the
    oT-reading items of the previous s-block must precede the round-LAG oT
    reset): r-broadcast matmuls, xT normalization and output projection of the
    PREVIOUS s-block (single K=128 matmul per 128x512 tile), and the q
    projection of the NEXT s-block.  (Schedules with flatter cross-s-block
    pipelining and an oT->SBUF staging copy simmed better but measured WORSE
    on HW — 398-434us vs 382us for this structure; keep this one.)
  - tails (end of each s-block): l=oT[64]; r=1/l via fast-reciprocal DVE op
    (SBUF-staged); consumed by the bc broadcast items early in the next
    s-block (h0 -> partitions 0:64, h1 -> 64:128 of a shared psum bank).
    PSUM: 2+2 score slots, oT_h0, oT_h1, q-proj bank, bc/outproj bank = 8.
  - host: sum the 8 partial outputs + bias terms.
"""

import sys

for _p in ("/opt/trn_rl_repo",):
    if _p not in sys.path:
        sys.path.insert(0, _p)

import numpy as np
import ml_dtypes

import concourse.bass as bass  # noqa: F401
import concourse.mybir as mybir
import concourse.tile as tile
from concourse import bacc
from concourse.bass_utils import run_bass_kernel_spmd

P = 128
S = 4096
D = 1024
H = 16
DK = 64
HL = 2            # heads per core
NC = 8            # cores
SB = 512          # s-block width
NSB = S // SB     # 8
TB = 128          # t-block (scores partition dim)
NTB = S // TB     # 32
DO = D // P       # 8 d-chunks


F32 = mybir.dt.float32
F16 = mybir.dt.float16
BF16 = mybir.dt.bfloat16
BF16_NP = ml_dtypes.bfloat16


def build_nc(reps: int = 1, loop_n: int = 0, phase1: bool = True, phase2: bool = True):
    """Build the per-core Bass kernel.

    `reps`: unrolled repetitions of the body.  `loop_n` > 0 instead wraps the
    body in a hardware For loop with that many iterations (timing builds).
    `phase1`/`phase2` gate the prologue / main loop for timing experiments.
    """
    from contextlib import ExitStack

    nc = bacc.Bacc("TRN2", target_bir_lowering=False, debug=False, num_devices=NC)
    qt = nc.dram_tensor("qt", [D, S], BF16, kind="ExternalInput").ap()
    kt = nc.dram_tensor("kt", [D, S], BF16, kind="ExternalInput").ap()
    vt = nc.dram_tensor("vt", [D, S], BF16, kind="ExternalInput").ap()
    wqk = nc.dram_tensor("wqk", [P, DO, 2, P], BF16, kind="ExternalInput").ap()
    wv = nc.dram_tensor("wv", [P, DO, P], BF16, kind="ExternalInput").ap()
    wot = nc.dram_tensor("wot", [P, D], BF16, kind="ExternalInput").ap()
    bqk = nc.dram_tensor("bqk", [P, 2], F32, kind="ExternalInput").ap()
    out = nc.dram_tensor("out", [S, D], F32, kind="ExternalOutput").ap()
    # The neuron NEFF cache keys on the HLO signature only (the Bass IR rides
    # out-of-band), so distinct kernel builds with identical I/O signatures can
    # collide with stale cache entries.  A version/config-sized dummy output
    # makes every build's signature unique.
    _KVER = 20
    vw = 16 + 64 * _KVER + 4 * loop_n + reps + (0 if phase1 else 1) + (0 if phase2 else 2)
    ver = nc.dram_tensor("ver", [1, vw], F32, kind="ExternalOutput").ap()

    qt_r = qt.rearrange("(o p) s -> p o s", p=P)
    kt_r = kt.rearrange("(o p) s -> p o s", p=P)
    vt_r = vt.rearrange("(o p) s -> p o s", p=P)
    out_r = out.rearrange("(so p) m -> p so m", p=P)  # [128, 32, 1024]

    with tile.TileContext(nc) as tc, ExitStack() as ctx:
        const = ctx.enter_context(tc.tile_pool(name="const", bufs=1))
        pers = ctx.enter_context(tc.tile_pool(name="pers", bufs=1))
        pin = ctx.enter_context(tc.tile_pool(name="pin", bufs=2))
        pP = ctx.enter_context(tc.tile_pool(name="pP", bufs=9))
        pout = ctx.enter_context(tc.tile_pool(name="pout", bufs=3))
        prr = ctx.enter_context(tc.tile_pool(name="prr", bufs=2))
        # PSUM (8 banks): slotA 2 + slotB 2 + oT0 1 + oT1 1 + proj 1 + bc/op 1
        psA = ctx.enter_context(tc.tile_pool(name="psA", bufs=1, space="PSUM"))
        psB = ctx.enter_context(tc.tile_pool(name="psB", bufs=1, space="PSUM"))
        psO0 = ctx.enter_context(tc.tile_pool(name="psO0", bufs=1, space="PSUM"))
        psO1 = ctx.enter_context(tc.tile_pool(name="psO1", bufs=1, space="PSUM"))
        psPj = ctx.enter_context(tc.tile_pool(name="psPj", bufs=1, space="PSUM"))
        psX = ctx.enter_context(tc.tile_pool(name="psX", bufs=1, space="PSUM"))

        wqk_sb = const.tile([P, DO, 2, P], BF16)
        nc.scalar.dma_start(wqk_sb[:], wqk)
        wv_sb = const.tile([P, DO, P], BF16)
        nc.scalar.dma_start(wv_sb[:], wv)
        wot_sb = const.tile([P, D], BF16)
        nc.sync.dma_start(wot_sb[:], wot)
        bqk_sb = const.tile([P, 2], F32)
        nc.sync.dma_start(bqk_sb[:], bqk)
        ones_sb = const.tile([1, DK], F32)
        nc.vector.memset(ones_sb[:], 1.0)
        ver_sb = const.tile([1, vw], F32)
        nc.vector.memset(ver_sb[:], float(vw))
        nc.sync.dma_start(ver, ver_sb[:])

        def body():
            qT = pers.tile([P, S], BF16, tag="qT", name="qT")
            # per-head zero-padded K=128 stationary tiles: zeros in the other
            # head's rows make QK a full-K matmul (HW: 163ns/MM vs 296 at K=64)
            # while the moving operand stays the combined [h0|h1] qT.
            kT0 = pers.tile([P, S], BF16, tag="kT0", name="kT0")
            kT1 = pers.tile([P, S], BF16, tag="kT1", name="kT1")
            vx = pers.tile([P, NTB, HL, DK + 1], F16, tag="vx", name="vx")
            xT = pers.tile([P, S], BF16, tag="xT", name="xT")
            nc.gpsimd.memset(kT0[DK:P, :], 0.0)
            nc.gpsimd.memset(kT1[0:DK, :], 0.0)
            nc.vector.memset(vx[:, :, :, DK], 1.0)

            def proj_q(sb):
                """DMA + project q for s-block sb -> qT[:, sb*SB:...] (as emit list)."""
                s0 = sb * SB
                items = []
                qs = pin.tile([P, DO, SB], BF16, tag="qs", name="qs")
                items.append(lambda: nc.sync.dma_start(qs[:], qt_r[:, :, s0 : s0 + SB]))
                pq = psPj.tile([P, SB], F32, tag="pj", name="pq")
                for o in range(DO):
                    items.append(
                        lambda o=o: nc.tensor.matmul(
                            pq, wqk_sb[:, o, 0, :], qs[:, o],
                            start=(o == 0), stop=(o == DO - 1),
                        )
                    )
                items.append(
                    lambda: nc.vector.tensor_scalar(
                        qT[:, s0 : s0 + SB], pq, bqk_sb[:, 0:1], 0.125,
                        mybir.AluOpType.add, mybir.AluOpType.mult,
                    )
                )
                return items

            # ---------------- prologue: k/v for full sequence, q for sb 0 ----
            if phase1:
                for sb in range(NSB):
                    s0 = sb * SB
                    ks = pin.tile([P, DO, SB], BF16, tag="ks", name="ks")
                    nc.sync.dma_start(ks[:], kt_r[:, :, s0 : s0 + SB])
                    vs = pin.tile([P, DO, SB], BF16, tag="vs", name="vs")
                    # ACT's HWDGE queue is idle in the prologue: run the v
                    # loads there, in parallel with the k chain on SP
                    nc.scalar.dma_start(vs[:], vt_r[:, :, s0 : s0 + SB])

                    pk = psPj.tile([P, SB], F32, tag="pj", name="pk")
                    for o in range(DO):
                        nc.tensor.matmul(
                            pk, wqk_sb[:, o, 1, :], ks[:, o],
                            start=(o == 0), stop=(o == DO - 1),
                        )
                    nc.vector.tensor_scalar(
                        kT0[0:DK, s0 : s0 + SB], pk[0:DK, :], bqk_sb[0:DK, 1:2], None,
                        mybir.AluOpType.add,
                    )
                    nc.vector.tensor_scalar(
                        kT1[DK:P, s0 : s0 + SB], pk[DK:P, :], bqk_sb[DK:P, 1:2], None,
                        mybir.AluOpType.add,
                    )

                    # v in [t, (h, dk)] layout; slots A/B are idle in the prologue
                    for tb in range(SB // TB):
                        pool = psA if tb % 2 == 0 else psB
                        tg = "sA" if tb % 2 == 0 else "sB"
                        pv = pool.tile([P, HL, SB], F32, tag=tg, name="pv")
                        pvv = pv[:, 0, 0:P]
                        for o in range(DO):
                            nc.tensor.matmul(
                                pvv, vs[:, o, tb * TB : (tb + 1) * TB], wv_sb[:, o],
                                start=(o == 0), stop=(o == DO - 1),
                            )
                        tbg = sb * (SB // TB) + tb
                        nc.vector.tensor_copy(vx[:, tbg, :, 0:DK], pvv[:])
                for it in proj_q(0):
                    it()

            # ---------------- steady state: flat pipeline over 256 rounds ---
            # Round r = (sb, tb): exp(sb, tb) reads the slot QK filled in round
            # r-1; QK(r+1) is emitted one round AHEAD so ACT never waits; PV
            # volleys run 1/round for tb 8..23 and 2/round for tb 24..31
            # (catch-up), so all 32 volleys of sb finish inside sb's rounds.
            # The previous s-block's tails/normalization/output projection and
            # the next s-block's q projection ride as paced work items.

            # ---------------- steady state: 8 s-blocks x 32 rounds ----------
            # Per round: QK both heads -> 2-bank slot (A/B ping-pong), one exp
            # [128, 1024] covers both heads, PV volleys lag LAG rounds.  The
            # previous s-block's bc/xT/outproj and the next s-block's q
            # projection ride as work items, one per round, emitted BEFORE the
            # PV volley so the oT-reading items stay ahead of the oT reset.
            LAG = 6
            for sb in range(NSB if phase2 else 0):
                s0 = sb * SB

                items = []
                if sb > 0:
                    pb = sb - 1
                    p0 = pb * SB
                    bc = psX.tile([P, SB], F32, tag="bc", name="bc")
                    for h, rt in enumerate(r_ts):
                        items.append(
                            lambda h=h, rt=rt: nc.tensor.matmul(
                                bc[h * DK : (h + 1) * DK, :], ones_sb[:], rt[:],
                                start=True, stop=True,
                            )
                        )
                    bc_sb = prr.tile([P, SB], F32, tag="bcs", name="bc_sb")
                    items.append(lambda: nc.vector.tensor_copy(bc_sb[:], bc))
                    for h, oT in enumerate(oTs):
                        items.append(
                            lambda h=h, oT=oT: nc.vector.tensor_tensor(
                                xT[h * DK : (h + 1) * DK, p0 : p0 + SB],
                                oT[0:DK, :], bc_sb[h * DK : (h + 1) * DK, :],
                                mybir.AluOpType.mult,
                            )
                        )

                    def outproj(si, mb, pb=pb):
                        so = pb * (SB // P) + si
                        m0 = mb * SB
                        op = psX.tile([P, SB], F32, tag="bc", name="op")
                        nc.tensor.matmul(
                            op, xT[:, so * P : (so + 1) * P], wot_sb[:, m0 : m0 + SB],
                            start=True, stop=True,
                        )
                        ob = pout.tile([P, SB], F32, tag="ob", name="ob")
                        nc.vector.tensor_copy(ob[:], op)
                        nc.sync.dma_start(out_r[:, so, m0 : m0 + SB], ob[:])

                    for si in range(SB // P):
                        for mb in range(2):
                            items.append(lambda si=si, mb=mb: outproj(si, mb))
                if sb < NSB - 1:
                    items.extend(proj_q(sb + 1))

                oT0 = psO0.tile([DK + 1, SB], F32, tag="oT0", name="oT0")
                oT1 = psO1.tile([DK + 1, SB], F32, tag="oT1", name="oT1")
                oTs = (oT0, oT1)
                pts = [None] * NTB

                def do_pv(tbl):
                    for h, oT in enumerate(oTs):
                        nc.tensor.matmul(
                            oT, vx[:, tbl, h, :], pts[tbl][:, h, :],
                            start=(tbl == 0), stop=(tbl == NTB - 1),
                        )

                for tb in range(NTB):
                    pool = psA if tb % 2 == 0 else psB
                    tg = "sA" if tb % 2 == 0 else "sB"
                    sc = pool.tile([P, HL, SB], F32, tag=tg, name="sc")
                    for h, kTh in enumerate((kT0, kT1)):
                        nc.tensor.matmul(
                            sc[:, h, :],
                            kTh[:, tb * TB : (tb + 1) * TB],
                            qT[:, s0 : s0 + SB],
                            start=True, stop=True,
                        )
                    pt = pP.tile([P, HL, SB], F16, tag="P", name="pt")
                    nc.scalar.activation(
                        pt[:], sc[:], mybir.ActivationFunctionType.Exp,
                    )
                    pts[tb] = pt
                    # items BEFORE do_pv: the oT-reading items (rounds 0..4)
                    # must precede the do_pv(0) reset at round LAG
                    if items:
                        items.pop(0)()
                    if tb >= LAG:
                        do_pv(tb - LAG)
                for tbl in range(NTB - LAG, NTB):
                    do_pv(tbl)
                for it in items:
                    it()

                # tails: r = 1/l per head (consumed by bc in the next s-block)
                r_ts = []
                for h, oT in enumerate(oTs):
                    l_t = prr.tile([1, SB], F32, tag=f"lt{h}", name="l_t")
                    r_t = prr.tile([1, SB], F32, tag=f"rt{h}", name="r_t")
                    r_s = prr.tile([1, SB], F32, tag=f"rs{h}", name="r_s")
                    # custom-DVE reciprocal mis-reads PSUM operands: stage in SBUF
                    nc.vector.tensor_copy(l_t[:], oT[DK : DK + 1, :])
                    nc.vector.reciprocal_approx_accurate(r_t[:], l_t[:], r_s[:])
                    r_ts.append(r_t)

            # final s-block's bc + xT + output projection (nothing to hide under)
            if phase2:
                pb = NSB - 1
                p0 = pb * SB
                bc = psX.tile([P, SB], F32, tag="bc", name="bc")
                for h, rt in enumerate(r_ts):
                    nc.tensor.matmul(
                        bc[h * DK : (h + 1) * DK, :], ones_sb[:], rt[:],
                        start=True, stop=True,
                    )
                bc_sb = prr.tile([P, SB], F32, tag="bcs", name="bc_sb")
                nc.vector.tensor_copy(bc_sb[:], bc)
                for h, oT in enumerate(oTs):
                    nc.vector.tensor_tensor(
                        xT[h * DK : (h + 1) * DK, p0 : p0 + SB],
                        oT[0:DK, :], bc_sb[h * DK : (h + 1) * DK, :],
                        mybir.AluOpType.mult,
                    )
                for si in range(SB // P):
                    so = pb * (SB // P) + si
                    for mb in range(2):
                        m0 = mb * SB
                        op = psX.tile([P, SB], F32, tag="bc", name="op")
                        nc.tensor.matmul(
                            op, xT[:, so * P : (so + 1) * P], wot_sb[:, m0 : m0 + SB],
                            start=True, stop=True,
                        )
                        ob = pout.tile([P, SB], F32, tag="ob", name="ob")
                        nc.vector.tensor_copy(ob[:], op)
                        nc.sync.dma_start(out_r[:, so, m0 : m0 + SB], ob[:])

        if loop_n > 0:
            with tc.For_i(0, loop_n, 1):
                body()
        else:
            for _ in range(reps):
                body()

    nc.finalize()
    return nc


def _pack_core_inputs(c, QT, KT, VT, Wq, bq, Wk, bk, Wv, Wo):
    """Per-core input dict (core c owns heads 2c, 2c+1)."""
    h0 = HL * c
    # [p, o, j] with j = h*64 + dk (both heads side by side in the M dim)
    wq = Wq[h0 : h0 + HL].reshape(HL, DO, P, DK).transpose(2, 1, 0, 3).reshape(P, DO, P)
    wk = Wk[h0 : h0 + HL].reshape(HL, DO, P, DK).transpose(2, 1, 0, 3).reshape(P, DO, P)
    wqk = np.stack([wq, wk], axis=2).astype(BF16_NP)  # [p, o, qk, j]
    wv = (
        Wv[h0 : h0 + HL].reshape(HL, DO, P, DK).transpose(2, 1, 0, 3).reshape(P, DO, P)
    ).astype(BF16_NP)
    wot = np.ascontiguousarray(
        Wo[:, h0 * DK : (h0 + HL) * DK].T
    ).astype(BF16_NP)  # [j, m]
    bqk = np.stack(
        [np.concatenate([bq[h0], bq[h0 + 1]]), np.concatenate([bk[h0], bk[h0 + 1]])],
        axis=1,
    ).astype(np.float32)  # [128, 2]
    return {
        "qt": QT, "kt": KT, "vt": VT,
        "wqk": np.ascontiguousarray(wqk),
        "wv": np.ascontiguousarray(wv),
        "wot": np.ascontiguousarray(wot),
        "bqk": np.ascontiguousarray(bqk),
    }


def make_in_maps(Q, K, V, Wq, bq, Wk, bk, Wv, bv, Wo, bo):
    QT = np.ascontiguousarray(Q.T).astype(BF16_NP)
    KT = np.ascontiguousarray(K.T).astype(BF16_NP)
    VT = np.ascontiguousarray(V.T).astype(BF16_NP)
    return [
        _pack_core_inputs(c, QT, KT, VT, Wq, bq, Wk, bk, Wv, Wo) for c in range(NC)
    ]


def host_combine(partials, Wq, bv, Wo, bo):
    total = np.zeros((S, D), np.float32)
    for p in partials:
        total += p
    # v-bias passes through softmax exactly as +bv on the concat features
    total += bv.reshape(-1).astype(np.float32) @ Wo.T.astype(np.float32) + bo
    return total


_NC_CACHE = {}


def _get_nc(reps=1):
    if reps not in _NC_CACHE:
        _NC_CACHE[reps] = build_nc(reps)
    return _NC_CACHE[reps]


def kernel(Q, K, V, Wq, bq, Wk, bk, Wv, bv, Wo, bo):
    args = [np.asarray(x) for x in (Q, K, V, Wq, bq, Wk, bk, Wv, bv, Wo, bo)]
    Q, K, V, Wq, bq, Wk, bk, Wv, bv, Wo, bo = args
    nc = _get_nc()
    in_maps = make_in_maps(Q, K, V, Wq, bq, Wk, bk, Wv, bv, Wo, bo)
    res = run_bass_kernel_spmd(nc, in_maps, core_ids=list(range(NC)))
    partials = [res.results[c]["out"] for c in range(NC)]
    return host_combine(partials, Wq, bv, Wo, bo)
